# revision 1
# baseline (speedup 1.0000x reference)
"""Causal GRN-EMA normalization kernel for 8x TRN2 NeuronCores (Bass/Tile).

Math (per batch b, channel c, time t):
    ema_t   = ALPHA*ema_{t-1} + (1-ALPHA)*x_t^2,  ema_{-1} = EMA_INIT
    ema_hat = ema_t / (1 - ALPHA^{t+1} + EPS)
    g       = sqrt(ema_hat + EPS)
    n       = g / (mean_c(g) + EPS)
    y       = gamma*(x*n) + beta + x

Strategy: data-parallel over B (16 batches -> 2 per core). The T-recurrence
is computed as a blocked scan on the tensor engine: for each block of
L=128 timesteps,
    within[i,c] = sum_{j<=i} (1-A)*A^(i-j) * x[j,c]^2     (lower-tri matmul)
    ema[i,c]    = within[i,c] + A^(i+1) * E_prev[c]       (K=1 outer matmul,
                                                           PSUM-accumulated)
    E_next[c]   = ema[L-1,c]                              (carry row)

x is pre-rotated on the host (partition p holds time (p-1) mod 128) so the
carry row lands on partition 0 (engines cannot address partition 127), and
the output is un-rotated on the host.
"""

import os
from contextlib import ExitStack

import numpy as np

ALPHA = 0.99
EPS = 1e-6
EMA_INIT = 1e-4

B, T, C = 16, 8192, 512
NCORES = 8
BPC = B // NCORES          # batches per core
L = 128                    # scan block (partition dim)
NBLK = T // L              # 64 blocks per batch

_MM_DTYPE = os.environ.get("KERNEL_MM_DTYPE", "f32r")  # "f32r" or "f32"

DEFAULT_CFG = dict(
    chunk=4,           # blocks per DMA chunk
    interleave=True,   # interleave the two batches' chunk streams
    ecopy="alt",       # "act" | "dve" | "alt" | "dma" | "dma_pool"
    ecopy_dve_every=3,  # for "alt": every Nth block's E-copy goes to DVE
    xin_bufs=6,
    bsq_bufs=3,
    g_bufs=4,
    ab_bufs=3,
    y_bufs=6,
    e_bufs=6,
    stat_bufs=8,
    psum_bufs=1,
    warmup_psum_shared=False,  # warmup matmuls use the main psum pool
    psum_per_block=True,       # per-block [128,512] psum tiles
    pblk_bufs=3,
    pool_y_stt=False,          # y-add as scalar_tensor_tensor on pool
    fold_meps=True,            # drop +EPS on the mean, fold 1/C into gamma
    square_pool_every=2,       # every Nth chunk's Square runs on pool (0=off)
    sttb_pool_every=0,         # NB: pool STT fails walrus ISA check — keep 0
    x_observer=True,
    host_beta=True,            # +beta applied on host during un-rotation
    ablate_dma=False,          # skip x/y DMAs (bound analysis only)
    ablate_compute=False,      # skip non-essential compute (bound analysis)
    hier=False,                # hierarchical chunk-level carry (kills the
                               # per-block PSUM->SBUF E-copy chain)
    wpool_bufs=2,
    eb_bufs=1,
    mean_pool=False,  # channel-sum via pool TSP+accum instead of ACT accum
    gt_observer=True,
    prefetch_head=2,  # DMA the first N chunks' x before the constants
    y_split=1,        # split the per-chunk y-out DMA into N pieces
)

_cache = {}


def _host_constants():
    # Partition rotation: partition p holds time index rot[p] = (p-1) mod L,
    # so the block-carry row (time L-1) lands on partition 0.
    i = np.arange(L, dtype=np.float64)
    # lhsT[j, i] = (1-A) * A^(i-j) for j <= i else 0  (within-scan weights)
    jj, ii = np.meshgrid(i, i, indexing="ij")
    lhsT = np.where(jj <= ii, (1.0 - ALPHA) * ALPHA ** (ii - jj), 0.0)
    rot = (np.arange(L) - 1) % L
    # Both matmul operands live in rotated partition order (x is pre-rotated
    # on host), so permute both axes of the lhsT.
    lmatT = lhsT[np.ix_(rot, rot)]
    # powv[0, p] = A^(rot[p]+1)
    powv = (ALPHA ** (i[rot] + 1))[None, :]
    # rden[p, k] = 1 / (1 - A^(128k + rot[p] + 1) + EPS)
    k = np.arange(NBLK, dtype=np.float64)
    tg = 128.0 * k[None, :] + i[rot][:, None] + 1.0
    rden = 1.0 / (1.0 - ALPHA**tg + EPS)
    # hierarchical-carry constants (chunk=4). a = per-block decay.
    a = ALPHA**L
    # wcolT[:, 5j + (j+1)] = carry-row weights (within_j at time L-1)
    wcolT = np.zeros((L, 20))
    for j in range(4):
        wcolT[:, 5 * j + (j + 1)] = lmatT[:, 0]
    # m2T[k, m]: D_1@32, D_2@64, D_3@96, D_4(next S)@0, over [S,w0,w1,w2,w3]
    m2T = np.zeros((5, L))
    for j, col in ((1, 32), (2, 64), (3, 96), (4, 0)):
        m2T[0, col] = a**j
        for m in range(j):
            m2T[1 + m, col] = a ** (j - 1 - m)
    sE = np.zeros((1, 5))
    sE[0, 0] = 1.0
    # powv replicated at partition bases 0/32/64/96 (PE tile_position rows)
    powv4 = np.zeros((L, L))
    for q in range(4):
        powv4[32 * q, :] = powv[0]
    return (
        np.ascontiguousarray(lmatT.astype(np.float32)),
        np.ascontiguousarray(powv.astype(np.float32)),
        np.ascontiguousarray(rden.astype(np.float32)),
        np.ascontiguousarray(wcolT.astype(np.float32)),
        np.ascontiguousarray(m2T.astype(np.float32)),
        np.ascontiguousarray(sE.astype(np.float32)),
        np.ascontiguousarray(powv4.astype(np.float32)),
    )


def _build_nc(repeat=1, cfg=None):
    import concourse.bacc as bacc
    import concourse.bass as bass
    import concourse.mybir as mybir
    import concourse.tile as tile

    cfg = {**DEFAULT_CFG, **(cfg or {})}
    CHUNK = cfg["chunk"]
    NCHUNK = NBLK // CHUNK
    assert NCHUNK * CHUNK == NBLK

    f32 = mybir.dt.float32
    mmdt = mybir.dt.float32r if _MM_DTYPE == "f32r" else mybir.dt.float32

    nc = bacc.Bacc()
    x_h = nc.dram_tensor("x", [BPC, T, C], f32, kind="ExternalInput")
    gamma_h = nc.dram_tensor("gamma", [1, C], f32, kind="ExternalInput")
    beta_h = nc.dram_tensor("beta", [1, C], f32, kind="ExternalInput")
    lmatT_h = nc.dram_tensor("lmatT", [L, L], mmdt, kind="ExternalInput")
    powv_h = nc.dram_tensor("powv", [1, L], mmdt, kind="ExternalInput")
    rden_h = nc.dram_tensor("rden", [L, NBLK], f32, kind="ExternalInput")
    einit_h = nc.dram_tensor("einit", [1, C], mmdt, kind="ExternalInput")
    wcolT_h = nc.dram_tensor("wcolT", [L, 20], mmdt, kind="ExternalInput")
    m2T_h = nc.dram_tensor("m2T", [5, L], mmdt, kind="ExternalInput")
    sE_h = nc.dram_tensor("sE", [1, 5], mmdt, kind="ExternalInput")
    powv4_h = nc.dram_tensor("powv4", [L, L], mmdt, kind="ExternalInput")
    y_h = nc.dram_tensor("y", [BPC, T, C], f32, kind="ExternalOutput")

    with tile.TileContext(nc) as tc, ExitStack() as ctx:
        singles = ctx.enter_context(tc.tile_pool(name="singles", bufs=1))
        xin = ctx.enter_context(tc.tile_pool(name="xin", bufs=cfg["xin_bufs"]))
        bsqp = ctx.enter_context(tc.tile_pool(name="bsqp", bufs=cfg["bsq_bufs"]))
        gp = ctx.enter_context(tc.tile_pool(name="gp", bufs=cfg["g_bufs"]))
        abp = ctx.enter_context(tc.tile_pool(name="abp", bufs=cfg["ab_bufs"]))
        yp = ctx.enter_context(tc.tile_pool(name="yp", bufs=cfg["y_bufs"]))
        ep = ctx.enter_context(tc.tile_pool(name="ep", bufs=cfg["e_bufs"]))
        statp = ctx.enter_context(tc.tile_pool(name="statp", bufs=cfg["stat_bufs"]))

        # --- head prefetch: start the first x transfers before anything ---
        CH = cfg["chunk"]
        prefetched = {}
        if cfg["prefetch_head"]:
            order = []
            if cfg["interleave"] and BPC == 2:
                for ci in range(NBLK // CH):
                    order += [(0, ci), (1, ci)]
            else:
                order = [(b, ci) for b in range(BPC) for ci in range(NBLK // CH)]
            for b0, c0 in order[: cfg["prefetch_head"]]:
                px = xin.tile([L, CH, C], f32, name=f"pf{b0}_{c0}", tag="xt")
                nc.sync.dma_start(
                    out=px,
                    in_=x_h[b0, c0 * CH * L : (c0 + 1) * CH * L, :].rearrange(
                        "(n p) c -> p n c", p=L
                    ),
                )
                prefetched[(b0, c0)] = px

        # --- constants, loaded once ---
        lmatT_s = singles.tile([L, L], mmdt)
        nc.sync.dma_start(out=lmatT_s, in_=lmatT_h[:, :])
        powv_s = singles.tile([1, L], mmdt)
        nc.sync.dma_start(out=powv_s, in_=powv_h[:, :])
        rden_s = singles.tile([L, NBLK], f32)
        nc.sync.dma_start(out=rden_s, in_=rden_h[:, :])
        # When fold_meps is on, kernel() ships gamma*C so rm = 1/s works
        # without the extra (s/C + EPS) tensor_scalar.
        gamma_s = singles.tile([L, C], f32)
        nc.sync.dma_start(
            out=gamma_s,
            in_=bass.AP(tensor=gamma_h, offset=0, ap=[[0, L], [1, C]]),
        )
        beta_s = singles.tile([L, C], f32)
        nc.sync.dma_start(
            out=beta_s,
            in_=bass.AP(tensor=beta_h, offset=0, ap=[[0, L], [1, C]]),
        )
        e_init = singles.tile([1, C], mmdt)
        nc.sync.dma_start(out=e_init, in_=einit_h[:, :])
        eps_s = singles.tile([L, 1], f32)
        nc.vector.memset(eps_s, EPS)
        hier = cfg["hier"]
        if hier:
            wcolT_s = singles.tile([L, 20], mmdt)
            nc.sync.dma_start(out=wcolT_s, in_=wcolT_h[:, :])
            m2T_s = singles.tile([5, L], mmdt)
            nc.sync.dma_start(out=m2T_s, in_=m2T_h[:, :])
            sE_s = singles.tile([1, 5], mmdt)
            nc.sync.dma_start(out=sE_s, in_=sE_h[:, :])
            powv4_s = singles.tile([L, L], mmdt)
            nc.sync.dma_start(out=powv4_s, in_=powv4_h[:, :])

        # Engine warm-ups: absorb the constant-DMA/memset waits into each
        # engine's vector clock (HW sync-wait slots per instruction are
        # extremely limited; Bacc legalizes overflow with event-semaphore
        # chains, but those cost latency in the steady state).
        wpsum = ctx.enter_context(tc.tile_pool(name="wpsum", bufs=1, space="PSUM"))
        warm = [
            (lmatT_s[:, :], lmatT_s[:, 0:1]),
            (powv_s[:, 0:1], powv_s[:, :]),
            (e_init[:, 0:L], e_init[:, 0:1]),
        ]
        if hier:
            warm.append((wcolT_s[:, 0:1], wcolT_s[:, 0:1]))
            warm.append((m2T_s[:, 0:1], m2T_s[:, 0:1]))
            warm.append((sE_s[:, :], sE_s[:, 0:1]))
            warm.append((powv4_s[:, 0:1], powv4_s[:, 0:1]))
        for wi, (wl, wr) in enumerate(warm):
            wup = wpsum.tile([L, L], f32, tag="warmup", name=f"wup{wi}")
            nc.tensor.matmul(
                wup[: wl.shape[-1], : wr.shape[-1]],
                wl.bitcast(f32), wr.bitcast(f32),
                start=True, stop=True,
            )
        if hier:
            psum = ctx.enter_context(
                tc.tile_pool(name="psum", bufs=cfg["pblk_bufs"], space="PSUM")
            )
            wpool = ctx.enter_context(
                tc.tile_pool(name="wpool", bufs=cfg["wpool_bufs"], space="PSUM")
            )
            ebpool = ctx.enter_context(
                tc.tile_pool(name="ebpool", bufs=cfg["eb_bufs"], space="PSUM")
            )
            esbp = ctx.enter_context(tc.tile_pool(name="esbp", bufs=3))
            swp = ctx.enter_context(tc.tile_pool(name="swp", bufs=3))
        elif cfg["psum_per_block"]:
            psum = ctx.enter_context(
                tc.tile_pool(name="psum", bufs=cfg["pblk_bufs"], space="PSUM")
            )
        else:
            psum = ctx.enter_context(
                tc.tile_pool(name="psum", bufs=cfg["psum_bufs"], space="PSUM")
            )
        scr_act = singles.tile([L, 1], f32)
        nc.scalar.copy(out=scr_act, in_=rden_s[:, 0:1])
        scr_act2 = singles.tile([L, 1], f32)
        nc.scalar.copy(out=scr_act2, in_=eps_s)
        scr_dve = singles.tile([L, 1], f32)
        nc.vector.tensor_copy(out=scr_dve, in_=gamma_s[:, 0:1])
        scr_pool = singles.tile([L, 1], f32)
        nc.gpsimd.tensor_copy(out=scr_pool, in_=beta_s[:, 0:1])
        obsp = ctx.enter_context(tc.tile_pool(name="obsp", bufs=2))

        # chunk schedule
        sched = []
        for _ in range(repeat):
            if cfg["interleave"] and BPC == 2:
                for ci in range(NCHUNK):
                    sched.append((0, ci))
                    sched.append((1, ci))
            else:
                for b in range(BPC):
                    for ci in range(NCHUNK):
                        sched.append((b, ci))

        e_cur = {}
        s_prev = {}
        blk_idx = 0
        ch_idx = 0
        for b, ci in sched:
            if ci == 0:
                e_cur[b] = e_init
                s_prev[b] = e_init[:, :]
            t0 = ci * CHUNK * L
            x_view = x_h[b, t0 : t0 + CHUNK * L, :].rearrange(
                "(n p) c -> p n c", p=L
            )
            y_view = y_h[b, t0 : t0 + CHUNK * L, :].rearrange(
                "(n p) c -> p n c", p=L
            )

            if (b, ci) in prefetched:
                xt = prefetched.pop((b, ci))
            else:
                xt = xin.tile([L, CHUNK, C], f32)
                if cfg["ablate_dma"]:
                    nc.sync.dma_start(
                        out=xt[0:1, 0, 0:1], in_=x_view[0:1, 0, 0:1]
                    )
                else:
                    nc.sync.dma_start(out=xt, in_=x_view)
            if cfg["x_observer"]:
                # DVE observer: cover the x-DMA semaphore on DVE's clock so
                # the per-block STT that reads xt keeps <=2 waits.
                obs = obsp.tile([1, 1], f32)
                nc.vector.tensor_copy(out=obs, in_=xt[0:1, 0, 0:1])

            # x^2 for the whole chunk in one op
            spe = cfg["square_pool_every"]
            bsq = bsqp.tile([L, CHUNK, C], mmdt)
            if cfg["ablate_compute"]:
                nc.scalar.activation(
                    out=bsq[0:1, 0, 0:1], in_=xt[0:1, 0, 0:1],
                    func=mybir.ActivationFunctionType.Square,
                )
            elif spe and (ch_idx % spe == 0):
                nc.gpsimd.tensor_mul(bsq, xt, xt)
            else:
                nc.scalar.activation(
                    out=bsq, in_=xt, func=mybir.ActivationFunctionType.Square
                )

            if hier:
                # chunk-level carry: D_j vectors for all 4 blocks in one shot
                pw = wpool.tile([5, C], f32)
                for j in range(CHUNK):
                    nc.tensor.matmul(
                        pw, wcolT_s[:, 5 * j : 5 * j + 5], bsq[:, j, :],
                        start=(j == 0), stop=False,
                    )
                nc.tensor.matmul(
                    pw, sE_s[:, :], s_prev[b], start=False, stop=True,
                )
                sw = swp.tile([5, C], mmdt)
                nc.scalar.copy(out=sw, in_=pw)
                eb = ebpool.tile([L, C], f32)
                nc.tensor.matmul(eb, m2T_s[:, :], sw, start=True, stop=True)
                e_sb = esbp.tile([L, C], mmdt)
                nc.scalar.copy(out=e_sb, in_=eb)
                # operand base partitions are limited to {0,32,64}; block 3's
                # carry (row 96) moves to its own base-0 tile
                e3_sb = esbp.tile([1, C], mmdt, tag="e3")
                nc.scalar.copy(out=e3_sb, in_=eb[96:97, :])

            per_blk = cfg["psum_per_block"]
            if not per_blk:
                pt = psum.tile([L, CHUNK, C], f32)
            gt = gp.tile([L, CHUNK, C], f32)
            yt = yp.tile([L, CHUNK, C], f32)
            # Pool observer: a dummy write into the fresh yt slot absorbs
            # the y-out DMA's slot-release semaphore on Pool's clock.
            nc.gpsimd.memset(yt[0:1, 0, 0:1], 0.0)
            if cfg["gt_observer"]:
                # ACT observer: dummy write into the fresh gt slot absorbs the
                # DVE slot-release wait, keeping the AP-bias Sqrt at 1 wait.
                nc.scalar.copy(out=gt[0:1, 0, 0:1], in_=eps_s[0:1, :])

            for j in range(CHUNK):
                kblk = ci * CHUNK + j
                if per_blk:
                    ptj = psum.tile([L, C], f32, tag="pblk", name=f"pb{blk_idx}")
                else:
                    ptj = pt[:, j, :]
                nc.tensor.matmul(
                    ptj, lmatT_s[:, :], bsq[:, j, :],
                    start=True, stop=False,
                )
                if hier:
                    if j == 0:
                        rhs_e = s_prev[b]
                        lhs_p = powv4_s[0:1, :]
                    elif j == 3:
                        rhs_e = e3_sb[:, :]
                        lhs_p = powv4_s[0:1, :]
                    else:
                        rhs_e = e_sb[32 * j : 32 * j + 1, :]
                        lhs_p = powv4_s[32 * j : 32 * j + 1, :]
                    e_next = None
                else:
                    rhs_e = e_cur[b][:, :]
                    lhs_p = powv_s[:, :]
                nc.tensor.matmul(
                    ptj, lhs_p, rhs_e,
                    start=False, stop=True,
                )
                if not hier:
                    # carry out: last row of ema (partition 0, rotated layout)
                    e_next = ep.tile([1, C], mmdt)
                    ec = cfg["ecopy"]
                    if ec == "dma":
                        nc.sync.dma_start(out=e_next, in_=ptj[0:1, :])
                    elif ec == "dma_pool":
                        nc.gpsimd.dma_start(out=e_next, in_=ptj[0:1, :])
                    elif ec == "act" or (
                        ec == "alt"
                        and (blk_idx % cfg["ecopy_dve_every"] != 0)
                    ):
                        nc.scalar.copy(out=e_next, in_=ptj[0:1, :])
                    else:
                        nc.vector.tensor_copy(out=e_next, in_=ptj[0:1, :])
                if cfg["ablate_compute"]:
                    nc.scalar.copy(out=gt[0:1, j, 0:1], in_=ptj[0:1, 0:1])
                    nc.vector.scalar_tensor_tensor(
                        out=yt[0:1, j, 0:1], in0=gt[0:1, j, 0:1], scalar=1.0,
                        in1=xt[0:1, j, 0:1],
                        op0=mybir.AluOpType.add, op1=mybir.AluOpType.mult,
                    )
                    if e_next is not None:
                        e_cur[b] = e_next
                    blk_idx += 1
                    continue
                # g = sqrt(ema * rden + EPS), s = sum_c g
                s = statp.tile([L, 1], f32)
                if cfg["mean_pool"]:
                    nc.scalar.activation(
                        out=gt[:, j, :],
                        in_=ptj,
                        func=mybir.ActivationFunctionType.Sqrt,
                        bias=eps_s,
                        scale=rden_s[:, kblk : kblk + 1],
                    )
                    mscr = abp.tile([L, C], f32, tag="mscr")
                    nc.gpsimd.tensor_scalar(
                        out=mscr, in0=gt[:, j, :], scalar1=1.0, scalar2=None,
                        op0=mybir.AluOpType.mult, accum_out=s,
                    )
                else:
                    nc.scalar.activation(
                        out=gt[:, j, :],
                        in_=ptj,
                        func=mybir.ActivationFunctionType.Sqrt,
                        bias=eps_s,
                        scale=rden_s[:, kblk : kblk + 1],
                        accum_out=s,
                    )
                if cfg["fold_meps"]:
                    # rm = 1/s; the /C is folded into gamma on the host
                    rm = statp.tile([L, 1], f32)
                    nc.vector.reciprocal(out=rm, in_=s)
                else:
                    # rm = 1 / (s/C + EPS)
                    sm = statp.tile([L, 1], f32)
                    nc.vector.tensor_scalar(
                        out=sm, in0=s, scalar1=1.0 / C, scalar2=EPS,
                        op0=mybir.AluOpType.mult, op1=mybir.AluOpType.add,
                    )
                    rm = statp.tile([L, 1], f32)
                    nc.vector.reciprocal(out=rm, in_=sm)
                # at = (g * rm) * gamma
                at = abp.tile([L, C], f32)
                nc.vector.scalar_tensor_tensor(
                    out=at, in0=gt[:, j, :], scalar=rm, in1=gamma_s,
                    op0=mybir.AluOpType.mult, op1=mybir.AluOpType.mult,
                )
                spb = cfg["sttb_pool_every"]
                beng = nc.gpsimd if (spb and blk_idx % spb == 0) else nc.vector
                if cfg["host_beta"]:
                    # y_dev = (at + 1) * x; +beta happens on the host
                    beng.scalar_tensor_tensor(
                        out=yt[:, j, :], in0=at, scalar=1.0, in1=xt[:, j, :],
                        op0=mybir.AluOpType.add, op1=mybir.AluOpType.mult,
                    )
                else:
                    # bt = (at + 1) * x
                    bt = abp.tile([L, C], f32)
                    beng.scalar_tensor_tensor(
                        out=bt, in0=at, scalar=1.0, in1=xt[:, j, :],
                        op0=mybir.AluOpType.add, op1=mybir.AluOpType.mult,
                    )
                    # y = bt + beta
                    if cfg["pool_y_stt"]:
                        nc.gpsimd.scalar_tensor_tensor(
                            out=yt[:, j, :], in0=bt, scalar=0.0, in1=beta_s,
                            op0=mybir.AluOpType.add, op1=mybir.AluOpType.add,
                        )
                    else:
                        nc.gpsimd.tensor_add(yt[:, j, :], bt, beta_s)
                if e_next is not None:
                    e_cur[b] = e_next
                blk_idx += 1

            # y stays rotated; host un-rotates
            if cfg["ablate_dma"]:
                nc.sync.dma_start(out=y_view[0:1, 0, 0:1], in_=yt[0:1, 0, 0:1])
            else:
                ys = cfg["y_split"]
                step = CHUNK // ys
                for p0 in range(0, CHUNK, step):
                    nc.sync.dma_start(
                        out=y_view[:, p0 : p0 + step, :],
                        in_=yt[:, p0 : p0 + step, :],
                    )
            if hier:
                s_prev[b] = e_sb[0:1, :]
            ch_idx += 1
    nc.finalize()
    return nc


def _get_nc():
    if "nc" not in _cache:
        _cache["nc"] = _build_nc()
    return _cache["nc"]


def kernel(x, gamma, beta, _want_profile=False):
    from concourse.bass_utils import run_bass_kernel_spmd

    x = np.asarray(x, dtype=np.float32)
    gamma = np.ascontiguousarray(np.asarray(gamma, dtype=np.float32))
    beta = np.ascontiguousarray(np.asarray(beta, dtype=np.float32))
    assert x.shape == (B, T, C), x.shape
    # pre-rotate: within each 128-step block, partition p holds time (p-1)%128
    x = np.roll(x.reshape(B, NBLK, L, C), 1, axis=2).reshape(B, T, C)

    lmatT, powv, rden, wcolT, m2T, sE, powv4 = _host_constants()
    einit = np.full((1, C), EMA_INIT, dtype=np.float32)
    nc = _get_nc()

    gamma_dev = gamma
    if DEFAULT_CFG["fold_meps"]:
        # device computes rm = 1/sum_c(g); fold the /C into gamma
        gamma_dev = np.ascontiguousarray(gamma * np.float32(C))

    in_maps = []
    for core in range(NCORES):
        xs = np.ascontiguousarray(x[core * BPC : (core + 1) * BPC])
        in_maps.append(
            {
                "x": xs,
                "gamma": gamma_dev,
                "beta": beta,
                "lmatT": lmatT,
                "powv": powv,
                "rden": rden,
                "einit": einit,
                "wcolT": wcolT,
                "m2T": m2T,
                "sE": sE,
                "powv4": powv4,
            }
        )

    # NOTE: trace=True requires antenv.axon_hooks, absent in this container.
    res = run_bass_kernel_spmd(nc, in_maps, list(range(NCORES)), trace=False)
    y = np.concatenate([res.results[core]["y"] for core in range(NCORES)], axis=0)
    # un-rotate (+beta if the device skipped it)
    y = np.roll(y.reshape(B, NBLK, L, C), -1, axis=2).reshape(B, T, C)
    if DEFAULT_CFG["host_beta"]:
        y = y + beta[None, :, :]
    y = np.ascontiguousarray(y)
    if _want_profile:
        _cache["last_profile"] = res
    return y



# revision 16
# speedup vs baseline: 1.4821x; 1.4821x over previous
"""Causal GRN-EMA normalization kernel for 8x TRN2 NeuronCores (Bass/Tile).

Math (per batch b, channel c, time t):
    ema_t   = ALPHA*ema_{t-1} + (1-ALPHA)*x_t^2,  ema_{-1} = EMA_INIT
    ema_hat = ema_t / (1 - ALPHA^{t+1} + EPS)
    g       = sqrt(ema_hat + EPS)
    n       = g / (mean_c(g) + EPS)
    y       = gamma*(x*n) + beta + x

Strategy (v2): data-parallel over B (16 batches -> 2 per core), bf16 I/O
(x shipped bf16, y returned bf16; rel tolerance 2e-2 >> bf16 rounding).
The T-recurrence is a blocked scan on the tensor engine, L=128 steps per
block, CHUNK=8 blocks per chunk:
    W_k[i,c]  = sum_{j<=i} (1-A)*A^(i-j) * x[128k+j,c]^2   (lower-tri matmul)
    D_k[c]    = W_k[127,c]                                  (within carry)
    E_k       = a*E_{k-1} + D_k,  a = A^128                 (block carry)
    ema[i,c]  = W_k[i,c] + A^(i+1) * E_{k-1}[c]
Carries are resolved chunk-wise without per-block PSUM->SBUF copies:
sw_m = [S_m; D_{8m..8m+7}] in one [9,C] PSUM tile (8 "wcol" matmuls pull
the D rows straight from bsq + 1 matmul computes S_m from sw_{m-1}); one
copy moves sw to SBUF, and each block's carry term is a K=9 matmul
lhsT9_j = outer(cvec_j, A^(i+1)) against sw (cvec_j resolves E_{8m+j-1}
linearly from sw_m).

x is pre-rotated on the host (partition p holds time (p-1) mod 128) so
the carry row lands on partition 0; output is un-rotated on the host.

Elementwise work is split to fit under the bf16 DMA roofline:
  ACT : per-block Sqrt(scale=rden, bias=eps) with accum_out -> s
        + a tunable share of the squares
  DVE : squares (2x bf16), u = g*gamma (2x), w1 = u*rm + 1 (4x),
        y = w1*x (2x), reciprocal
  Pool: sw copies, observers
"""

import os
from contextlib import ExitStack

import numpy as np

ALPHA = 0.99
EPS = 1e-6
EMA_INIT = 1e-4

B, T, C = 16, 8192, 512
NCORES = 8
BPC = B // NCORES          # batches per core
L = 128                    # scan block (partition dim)
NBLK = T // L              # 64 blocks per batch

DEFAULT_CFG = dict(
    chunk=4,            # blocks per chunk
    interleave=True,    # interleave the two batches' chunk streams
    sq_act_every=0,     # every Nth chunk's Square runs on ACT (0=never)
    sq_pool_num=6,      # squares: Pool gets num of every den chunks
    sq_pool_den=8,
    sw_engine="act",    # engine for the [CH+1,C] PSUM->SBUF sw copy
    y_dma="sp",         # engine issuing y DMAs
    tail_split=2,       # recip/u/w1/y granularity: CH//tail_split blocks
    split_mm=True,      # emit all withins before resolves per chunk
    s_after_within=False,  # emit the S matmul after the withins
    xin_bufs=8,
    bsq_bufs=3,
    g_bufs=3,
    u_bufs=2,
    w_bufs=2,
    y_bufs=3,
    sw_bufs=3,
    stat_bufs=4,
    pblk_bufs=6,
    pw_bufs=2,
    prefetch_head=5,
    y_split=1,
    x_observer=True,
)

_cache = {}


def _host_constants(CH):
    i = np.arange(L, dtype=np.float64)
    jj, ii = np.meshgrid(i, i, indexing="ij")
    lmat = np.where(jj <= ii, (1.0 - ALPHA) * ALPHA ** (ii - jj), 0.0)
    rot = (np.arange(L) - 1) % L
    lmatT = lmat[np.ix_(rot, rot)]            # both axes rotated
    powv = ALPHA ** (i + 1)                   # unrotated time i
    powv_rot = powv[rot]
    # rden[p, k] = 1 / (1 - A^(128k + rot[p] + 1) + EPS)
    k = np.arange(NBLK, dtype=np.float64)
    tg = 128.0 * k[None, :] + i[rot][:, None] + 1.0
    rden = 1.0 / (1.0 - ALPHA**tg + EPS)
    a = ALPHA**L
    # wcolT[:, (CH+1)*j + (1+j)] = rotated D-row weights for block j
    wcolT = np.zeros((L, (CH + 1) * CH))
    for j in range(CH):
        wcolT[:, (CH + 1) * j + (1 + j)] = lmatT[:, 0]
    # cvec_j resolves E entering block j of a chunk from sw rows
    cvec = np.zeros((CH + 1, CH + 1))
    for j in range(CH + 1):
        cvec[j, 0] = a**j
        for m in range(j):
            cvec[j, 1 + m] = a ** (j - 1 - m)
    # lhsT9 stack: [CH+1, CH*L]; slice j = outer(cvec_j, powv_rot)
    lhsT9 = np.zeros((CH + 1, CH * L))
    for j in range(CH):
        lhsT9[:, j * L : (j + 1) * L] = np.outer(cvec[j], powv_rot)
    # S-matmul lhsT: [CH+1, 2*(CH+1)]; col 0 = scoef (steady), col CH+1 = e0
    sS = np.zeros((CH + 1, 2 * (CH + 1)))
    sS[:, 0] = cvec[CH]
    sS[0, CH + 1] = 1.0
    return (
        np.ascontiguousarray(lmatT),
        np.ascontiguousarray(rden),
        np.ascontiguousarray(wcolT),
        np.ascontiguousarray(lhsT9),
        np.ascontiguousarray(sS),
    )


def _build_nc(cfg=None):
    import concourse.bacc as bacc
    import concourse.bass as bass
    import concourse.mybir as mybir
    import concourse.tile as tile

    cfg = {**DEFAULT_CFG, **(cfg or {})}
    CH = cfg["chunk"]
    NCHUNK = NBLK // CH
    assert NCHUNK * CH == NBLK

    f32 = mybir.dt.float32
    bf16 = mybir.dt.bfloat16

    nc = bacc.Bacc()
    x_h = nc.dram_tensor("x", [BPC, T, C], bf16, kind="ExternalInput")
    gamma_h = nc.dram_tensor("gamma", [1, C], bf16, kind="ExternalInput")
    lmatT_h = nc.dram_tensor("lmatT", [L, L], bf16, kind="ExternalInput")
    rden_h = nc.dram_tensor("rden", [L, NBLK], f32, kind="ExternalInput")
    wcolT_h = nc.dram_tensor("wcolT", [L, (CH + 1) * CH], bf16, kind="ExternalInput")
    lhsT9_h = nc.dram_tensor("lhsT9", [CH + 1, CH * L], bf16, kind="ExternalInput")
    sS_h = nc.dram_tensor("sS", [CH + 1, 2 * (CH + 1)], bf16, kind="ExternalInput")
    einit9_h = nc.dram_tensor("einit9", [CH + 1, C], bf16, kind="ExternalInput")
    y_h = nc.dram_tensor("y", [BPC, T, C], bf16, kind="ExternalOutput")

    with tile.TileContext(nc) as tc, ExitStack() as ctx:
        singles = ctx.enter_context(tc.tile_pool(name="singles", bufs=1))
        xin = ctx.enter_context(tc.tile_pool(name="xin", bufs=cfg["xin_bufs"]))
        bsqp = ctx.enter_context(tc.tile_pool(name="bsqp", bufs=cfg["bsq_bufs"]))
        gp = ctx.enter_context(tc.tile_pool(name="gp", bufs=cfg["g_bufs"]))
        up = ctx.enter_context(tc.tile_pool(name="up", bufs=cfg["u_bufs"]))
        wp = ctx.enter_context(tc.tile_pool(name="wp", bufs=cfg["w_bufs"]))
        yp = ctx.enter_context(tc.tile_pool(name="yp", bufs=cfg["y_bufs"]))
        swp = ctx.enter_context(tc.tile_pool(name="swp", bufs=cfg["sw_bufs"]))
        statp = ctx.enter_context(tc.tile_pool(name="statp", bufs=cfg["stat_bufs"]))
        obsp = ctx.enter_context(tc.tile_pool(name="obsp", bufs=2))

        # --- head prefetch: start the first x transfers before constants ---
        prefetched = {}
        order = []
        if cfg["interleave"] and BPC == 2:
            for ci in range(NCHUNK):
                order += [(0, ci), (1, ci)]
        else:
            order = [(b, ci) for b in range(BPC) for ci in range(NCHUNK)]
        for b0, c0 in order[: cfg["prefetch_head"]]:
            px = xin.tile([L, CH, C], bf16, name=f"pf{b0}_{c0}", tag="xt")
            nc.sync.dma_start(
                out=px,
                in_=x_h[b0, c0 * CH * L : (c0 + 1) * CH * L, :].rearrange(
                    "(n p) c -> p n c", p=L
                ),
            )
            prefetched[(b0, c0)] = px

        # --- constants ---
        lmatT_s = singles.tile([L, L], bf16)
        nc.sync.dma_start(out=lmatT_s, in_=lmatT_h[:, :])
        rden_s = singles.tile([L, NBLK], f32)
        nc.sync.dma_start(out=rden_s, in_=rden_h[:, :])
        wcolT_s = singles.tile([L, (CH + 1) * CH], bf16)
        nc.sync.dma_start(out=wcolT_s, in_=wcolT_h[:, :])
        lhsT9_s = singles.tile([CH + 1, CH * L], bf16)
        nc.sync.dma_start(out=lhsT9_s, in_=lhsT9_h[:, :])
        sS_s = singles.tile([CH + 1, 2 * (CH + 1)], bf16)
        nc.sync.dma_start(out=sS_s, in_=sS_h[:, :])
        einit9_s = singles.tile([CH + 1, C], bf16)
        nc.sync.dma_start(out=einit9_s, in_=einit9_h[:, :])
        # gamma*C replicated across the chunk free dim: [L, CH, C]
        gamma_s = singles.tile([L, CH, C], bf16)
        nc.sync.dma_start(
            out=gamma_s,
            in_=bass.AP(tensor=gamma_h, offset=0, ap=[[0, L], [0, CH], [1, C]]),
        )
        eps_s = singles.tile([L, 1], f32)
        nc.vector.memset(eps_s, EPS)

        # Engine warm-ups: absorb constant-DMA waits into each engine's
        # vector clock before the steady state.
        pwpool = ctx.enter_context(
            tc.tile_pool(name="pwpool", bufs=cfg["pw_bufs"], space="PSUM")
        )
        warm = [
            (lmatT_s[:, 0:2], lmatT_s[:, 0:1]),
            (wcolT_s[:, 0:2], wcolT_s[:, 0:1]),
            (lhsT9_s[:, 0:2], lhsT9_s[:, 0:1]),
            (sS_s[:, 0:2], sS_s[:, 0:1]),
            (einit9_s[:, 0:2], einit9_s[:, 0:1]),
        ]
        for wi, (wl, wr) in enumerate(warm):
            wup = pwpool.tile([CH + 1, C], f32, tag="pw", name=f"wup{wi}")
            nc.tensor.matmul(
                wup[: wl.shape[-1], : wr.shape[-1]], wl, wr,
                start=True, stop=True,
            )
        scr_act = singles.tile([L, 1], f32)
        nc.scalar.copy(out=scr_act, in_=rden_s[:, 0:1])
        scr_dve = singles.tile([L, 1], f32)
        nc.vector.tensor_copy(out=scr_dve, in_=eps_s)
        scr_pool = singles.tile([L, 1], f32)
        nc.gpsimd.tensor_copy(out=scr_pool, in_=eps_s)
        scr_pool2 = singles.tile([1, 4], bf16)
        nc.gpsimd.tensor_copy(out=scr_pool2, in_=gamma_s[0:1, 0, 0:4])

        psum = ctx.enter_context(
            tc.tile_pool(name="psum", bufs=cfg["pblk_bufs"], space="PSUM")
        )

        sched = []
        for b, ci in order:
            sched.append((b, ci))

        sw_prev = {}
        blk_idx = 0
        ch_idx = 0
        for b, ci in sched:
            t0 = ci * CH * L
            x_view = x_h[b, t0 : t0 + CH * L, :].rearrange("(n p) c -> p n c", p=L)
            y_view = y_h[b, t0 : t0 + CH * L, :].rearrange("(n p) c -> p n c", p=L)

            if (b, ci) in prefetched:
                xt = prefetched.pop((b, ci))
            else:
                xt = xin.tile([L, CH, C], bf16, tag="xt")
                nc.sync.dma_start(out=xt, in_=x_view)
            if cfg["x_observer"]:
                obs = obsp.tile([1, 1], bf16)
                nc.vector.tensor_copy(out=obs, in_=xt[0:1, 0, 0:1])

            # squares for the whole chunk
            sae = cfg["sq_act_every"]
            bsq = bsqp.tile([L, CH, C], bf16)
            if sae and (ch_idx % sae == 0):
                nc.scalar.activation(
                    out=bsq, in_=xt, func=mybir.ActivationFunctionType.Square
                )
            elif (ch_idx % cfg["sq_pool_den"]) < cfg["sq_pool_num"]:
                nc.gpsimd.tensor_mul(bsq, xt, xt)
            else:
                nc.vector.tensor_mul(bsq, xt, xt)

            # chunk-state sw_m = [S_m; D rows] in PSUM, then one SBUF copy.
            # Emitted first so the sw chain resolves as early as possible;
            # the within matmuls below can fill PE gaps.
            pw = pwpool.tile([CH + 1, C], f32, tag="pw")
            for j in range(CH):
                nc.tensor.matmul(
                    pw, wcolT_s[:, (CH + 1) * j : (CH + 1) * (j + 1)],
                    bsq[:, j, :],
                    start=(j == 0), stop=False,
                )
            if ci == 0:
                rhs_S = einit9_s[:, :]
                lhs_S = sS_s[:, CH + 1 : 2 * (CH + 1)]
            else:
                rhs_S = sw_prev[b][:, :]
                lhs_S = sS_s[:, 0 : CH + 1]
            if not cfg["s_after_within"]:
                nc.tensor.matmul(pw, lhs_S, rhs_S, start=False, stop=True)

            ptiles = []
            for j in range(CH):
                ptj = psum.tile([L, C], f32, tag="pblk", name=f"pb{blk_idx+j}")
                ptiles.append(ptj)
                if cfg["split_mm"]:
                    nc.tensor.matmul(
                        ptj, lmatT_s[:, :], bsq[:, j, :], start=True, stop=False
                    )
            if cfg["s_after_within"]:
                nc.tensor.matmul(pw, lhs_S, rhs_S, start=False, stop=True)
            sw = swp.tile([CH + 1, C], bf16)
            sweng = cfg["sw_engine"]
            if sweng == "mix":
                sweng = "pool" if ch_idx % 2 == 0 else "act"
            elif sweng == "actdve":
                sweng = "dve" if ch_idx % 4 == 0 else "act"
            if sweng == "pool":
                nc.gpsimd.tensor_copy(out=sw, in_=pw)
            elif sweng == "dve":
                nc.vector.tensor_copy(out=sw, in_=pw)
            else:
                nc.scalar.copy(out=sw, in_=pw)
            sw_prev[b] = sw

            gt = gp.tile([L, CH, C], bf16)
            s_ch = statp.tile([L, CH], f32, tag="s")
            for j in range(CH):
                kblk = ci * CH + j
                ptj = ptiles[j]
                if not cfg["split_mm"]:
                    nc.tensor.matmul(
                        ptj, lmatT_s[:, :], bsq[:, j, :], start=True, stop=False
                    )
                nc.tensor.matmul(
                    ptj, lhsT9_s[:, j * L : (j + 1) * L], sw[:, :],
                    start=False, stop=True,
                )
                # g = sqrt(ema*rden + EPS) in bf16; s = sum_c g in f32
                nc.scalar.activation(
                    out=gt[:, j, :],
                    in_=ptj,
                    func=mybir.ActivationFunctionType.Sqrt,
                    bias=eps_s,
                    scale=rden_s[:, kblk : kblk + 1],
                    accum_out=s_ch[:, j : j + 1],
                )
            blk_idx += CH

            # tail: rm = 1/s, u = g*gamma, w1 = u*rm + 1, y = w1*x,
            # processed in CH//tail_split-block groups to spread DVE work
            ts = cfg["tail_split"]
            GR = CH // ts
            rm_ch = statp.tile([L, CH], f32, tag="rm")
            ut = up.tile([L, CH, C], bf16)
            w1 = wp.tile([L, CH, C], bf16)
            yt = yp.tile([L, CH, C], bf16)
            for g0 in range(0, CH, GR):
                nc.vector.reciprocal(
                    out=rm_ch[:, g0 : g0 + GR], in_=s_ch[:, g0 : g0 + GR]
                )
                nc.vector.tensor_mul(
                    ut[:, g0 : g0 + GR, :], gt[:, g0 : g0 + GR, :],
                    gamma_s[:, g0 : g0 + GR, :],
                )
                for j in range(g0, g0 + GR):
                    nc.vector.tensor_scalar(
                        out=w1[:, j, :], in0=ut[:, j, :],
                        scalar1=rm_ch[:, j : j + 1], scalar2=1.0,
                        op0=mybir.AluOpType.mult, op1=mybir.AluOpType.add,
                    )
                nc.vector.tensor_mul(
                    yt[:, g0 : g0 + GR, :], w1[:, g0 : g0 + GR, :],
                    xt[:, g0 : g0 + GR, :],
                )
                y_eng = {
                    "pool": nc.gpsimd,
                    "dve": nc.vector,
                    "act": nc.scalar,
                    "sp": nc.sync,
                }[cfg["y_dma"]]
                y_eng.dma_start(
                    out=y_view[:, g0 : g0 + GR, :],
                    in_=yt[:, g0 : g0 + GR, :],
                )
            ch_idx += 1
    nc.finalize()
    return nc


def _get_nc():
    if "nc" not in _cache:
        _cache["nc"] = _build_nc()
    return _cache["nc"]


def kernel(x, gamma, beta, _want_profile=False):
    import ml_dtypes
    from concourse.bass_utils import run_bass_kernel_spmd

    bf = ml_dtypes.bfloat16
    x = np.asarray(x, dtype=np.float32)
    gamma = np.ascontiguousarray(np.asarray(gamma, dtype=np.float32))
    beta = np.ascontiguousarray(np.asarray(beta, dtype=np.float32))
    assert x.shape == (B, T, C), x.shape
    CH = DEFAULT_CFG["chunk"]
    # pre-rotate: within each 128-step block, partition p holds time (p-1)%128
    xr = np.roll(x.reshape(B, NBLK, L, C), 1, axis=2).reshape(B, T, C)
    xb = xr.astype(bf)

    lmatT, rden, wcolT, lhsT9, sS = _host_constants(CH)
    einit9 = np.zeros((CH + 1, C), dtype=np.float32)
    einit9[0, :] = EMA_INIT
    nc = _get_nc()

    gamma_dev = (gamma * np.float32(C)).astype(bf)  # rm = 1/sum_c g; /C folded

    consts = {
        "gamma": np.ascontiguousarray(gamma_dev),
        "lmatT": np.ascontiguousarray(lmatT.astype(bf)),
        "rden": np.ascontiguousarray(rden.astype(np.float32)),
        "wcolT": np.ascontiguousarray(wcolT.astype(bf)),
        "lhsT9": np.ascontiguousarray(lhsT9.astype(bf)),
        "sS": np.ascontiguousarray(sS.astype(bf)),
        "einit9": np.ascontiguousarray(einit9.astype(bf)),
    }
    in_maps = []
    for core in range(NCORES):
        xs = np.ascontiguousarray(xb[core * BPC : (core + 1) * BPC])
        in_maps.append({"x": xs, **consts})

    res = run_bass_kernel_spmd(nc, in_maps, list(range(NCORES)), trace=False)
    y = np.concatenate([res.results[core]["y"] for core in range(NCORES)], axis=0)
    y = np.roll(y.reshape(B, NBLK, L, C), -1, axis=2).reshape(B, T, C)
    y = y.astype(np.float32) + beta[None, :, :]
    y = np.ascontiguousarray(y)
    if _want_profile:
        _cache["last_profile"] = res
    return y
